# revision 23
# baseline (speedup 1.0000x reference)
"""XL-BOMD rank-4 Krylov propagation (EnergyXL) on 8 TRN2 NeuronCores.

Data-parallel over molecules: 512 mols -> 64 per core, processed in
pairs.  Per molecule (N=192, rank=4) the reference computes

    out = -V (W^T W)^{-1} W^T dDS,   W = F(V) = R V R - V

over the Gram-Schmidt basis V of the Krylov space K_4(dDS).  The
output is invariant under ANY invertible change of basis of K_4
(W is linear in V), so we use the raw power iterates S_k = R^k dDS R^k
directly:

    S_0 = D - P;  S_k = R S_{k-1} R              (8 bf16 PE products/mol)
    g[s] = <S_a, S_b>  (a+b = s, s = 0..8)       (Gram is Hankel: 9 ips)
    O[I,J] = g[I+J+2] - 2 g[I+J+1] + g[I+J],  c[J] = g[J+1] - g[J]
    y = O^{-1} c   (batched 4x4 Gauss over 32-mol blocks)
    out = -sum_I y_I S_I                         (fused scale-add chain)

Layout per pair (A, B): hi tiles [128, 384] (A rows 0:128 in cols
0:192, B in 192:384), lo tiles [128, 192] (A rows 128:192 in
partitions 0:64, B in 64:128).  Matmuls run in bf16 (PSUM fp32).
Gram inner products: DVE/Pool elementwise multiply (bf16 2x) then a
one-hot selector matmul on the PE column-sums each product into a
per-molecule row of a PSUM accumulator; a per-level tensor_reduce
lands g directly in the [32, 9] solver layout.
"""

import sys

sys.path.insert(0, "/opt/trn_rl_repo")

import numpy as np

import concourse.bass as bass
import concourse.bacc as bacc
import concourse.tile as tile
from concourse import mybir
from concourse.bass_utils import run_bass_kernel_spmd

F32 = mybir.dt.float32
BF16 = mybir.dt.bfloat16
ALU = mybir.AluOpType
ACTF = mybir.ActivationFunctionType

NMOL, N, RANK = 512, 192, 4
NCORES = 8
MPC = NMOL // NCORES      # 64 molecules per core
NPAIR = MPC // 2          # 32 pairs
BLKP = 8                  # pairs per block (16 mols -> one batched solve)
NBLK = NPAIR // BLKP
HI, LO = 128, 64
PAIR_OFF = 4 * 2 * BLKP   # ybc column offset of the packed-lo pair values

# g[s] = <S_a, S_b> with a+b = s; level k (k=1..4) computes s = 2k-1, 2k.
G_PAIRS = {0: (0, 0), 1: (0, 1), 2: (1, 1), 3: (1, 2), 4: (2, 2),
           5: (2, 3), 6: (3, 3), 7: (3, 4), 8: (4, 4)}


def build_core_kernel():
    nc = bacc.Bacc(None, target_bir_lowering=False, enable_partition_id=False)
    D = nc.dram_tensor("D", [MPC, N, N], F32, kind="ExternalInput")
    P = nc.dram_tensor("P", [MPC, N, N], F32, kind="ExternalInput")
    R = nc.dram_tensor("Rm", [MPC, N, N], F32, kind="ExternalInput")
    OUT = nc.dram_tensor("OUT", [MPC, N, N], F32, kind="ExternalOutput")
    with tile.TileContext(nc) as tc:
        _body(nc, tc, D, P, R, OUT)
    nc.finalize()
    return nc


def _consts(nc, pool):
    c = {}
    # cb1: one-hot ones-column selector bank (col 31 = all-ones, fp32);
    # window cb1[:, 31-r : 63-r] routes a partials column-sum to gather
    # row r.
    W = 2 * BLKP
    cb1 = pool.tile([HI, 2 * W - 1], F32)
    nc.vector.memset(cb1, 0.0)
    nc.vector.memset(cb1[:, W - 1:W], 1.0)
    c["cb1"] = cb1
    # cb2: col 31 = upper-half ones, col 32 = lower-half ones; window at
    # row r sends partitions 0:64 to row r and 64:128 to row r+1.
    cb2 = pool.tile([HI, 2 * W], F32)
    nc.vector.memset(cb2, 0.0)
    nc.vector.memset(cb2[0:LO, W - 1:W], 1.0)
    nc.vector.memset(cb2[LO:HI, W:W + 1], 1.0)
    c["cb2"] = cb2
    ones = pool.tile([HI, HI], F32)
    nc.vector.memset(ones, 1.0)
    c["ones"] = ones
    # selp: even partitions -> ones in cols 0:64, odd -> ones in 64:128.
    idp = pool.tile([HI, 1], mybir.dt.int32)
    nc.gpsimd.iota(idp, pattern=[[0, 1]], base=0, channel_multiplier=1)
    podd_i = pool.tile([HI, 1], mybir.dt.int32)
    nc.vector.tensor_scalar(out=podd_i, in0=idp, scalar1=1, scalar2=None,
                            op0=ALU.bitwise_and)
    podd = pool.tile([HI, 1], F32)
    nc.vector.tensor_scalar(out=podd, in0=podd_i, scalar1=1.0, scalar2=None,
                            op0=ALU.mult)
    pevn = pool.tile([HI, 1], F32)
    nc.vector.tensor_scalar(out=pevn, in0=podd, scalar1=-1.0, scalar2=1.0,
                            op0=ALU.mult, op1=ALU.add)
    selp = pool.tile([HI, HI], F32)
    nc.vector.tensor_scalar(out=selp[:, 0:LO], in0=ones[:, 0:LO],
                            scalar1=pevn, scalar2=None, op0=ALU.mult)
    nc.vector.tensor_scalar(out=selp[:, LO:HI], in0=ones[:, 0:LO],
                            scalar1=podd, scalar2=None, op0=ALU.mult)
    c["selp"] = selp
    # mask32[c, 4m+I] = (c == m); mask2[c, 4j+I] = (c in {2j, 2j+1})
    nm = 2 * BLKP
    mi = pool.tile([nm, 4 * nm], mybir.dt.int32)
    nc.gpsimd.iota(mi, pattern=[[-1, nm], [0, 4]], base=0,
                   channel_multiplier=1)
    mask32 = pool.tile([nm, 4 * nm], F32)
    nc.vector.tensor_scalar(out=mask32, in0=mi, scalar1=0, scalar2=None,
                            op0=ALU.is_equal)
    c["mask32"] = mask32
    mj = pool.tile([nm, 4 * BLKP], mybir.dt.int32)
    nc.gpsimd.iota(mj, pattern=[[-2, BLKP], [0, 4]], base=0,
                   channel_multiplier=1)
    m20 = pool.tile([nm, 4 * BLKP], F32)
    nc.vector.tensor_scalar(out=m20, in0=mj, scalar1=0, scalar2=None,
                            op0=ALU.is_equal)
    m21 = pool.tile([nm, 4 * BLKP], F32)
    nc.vector.tensor_scalar(out=m21, in0=mj, scalar1=1, scalar2=None,
                            op0=ALU.is_equal)
    mask2 = pool.tile([nm, 4 * BLKP], F32)
    nc.vector.tensor_add(mask2, m20, m21)
    c["mask2"] = mask2
    return c


def _body(nc, tc, D, P, R, OUT):
    import contextlib

    ctx = contextlib.ExitStack()
    with ctx:
        cpool = ctx.enter_context(tc.tile_pool(name="consts", bufs=1))
        sp = ctx.enter_context(tc.tile_pool(name="sp", bufs=BLKP + 2))
        tmp = ctx.enter_context(tc.tile_pool(name="tmp", bufs=3))
        tail = ctx.enter_context(tc.tile_pool(name="tail", bufs=2))
        ps_big = ctx.enter_context(tc.tile_pool(name="ps_big", bufs=3,
                                                space="PSUM"))
        ps_lo = ctx.enter_context(tc.tile_pool(name="ps_lo", bufs=3,
                                               space="PSUM"))
        ps_tl = ctx.enter_context(tc.tile_pool(name="ps_tl", bufs=1,
                                               space="PSUM"))
        C = _consts(nc, cpool)

        def emit_tail(bprev, stprev, pairsprev):
            g_sb = _gather(nc, stprev, pairsprev, C, tail, ps_tl)
            ybc = _tail(nc, pairsprev, C, tail, ps_tl, g_sb)
            for q in pairsprev:
                _combo(nc, stprev[q], q - bprev * BLKP, ybc, tmp)
            for q in pairsprev:
                _store(nc, OUT, q, stprev[q])

        prev = None
        for b in range(NBLK):
            pairs = list(range(b * BLKP, (b + 1) * BLKP))
            st = {}
            for q in pairs:
                st[q] = _load_prep(nc, D, P, R, q, sp, tmp)
            # gram for level k-1 is emitted after level k's products so the
            # ACT squares queue behind the drains they'd otherwise delay;
            # the previous block's tail is deferred past this block's
            # level 1 so its solve/combo overlap these products.
            for k in range(1, RANK + 1):
                _level(nc, st, pairs, k, sp, tmp, ps_big, ps_lo)
                _gram_level(nc, st, pairs,
                            [0] if k == 1 else [2 * k - 3, 2 * k - 2], tmp)
                if k == 1 and prev is not None:
                    emit_tail(*prev)
            _gram_level(nc, st, pairs, [2 * RANK - 1, 2 * RANK], tmp)
            prev = (b, st, pairs)
        emit_tail(*prev)


def _load_prep(nc, D, P, R, q, sp, tmp):
    """DMA loads, dDS = D - P (bf16), R cast for one pair."""
    mA, mB = 2 * q, 2 * q + 1
    stg = {}
    for nm, T in (("d", D), ("p", P), ("r", R)):
        sh = tmp.tile([HI, 2 * N], F32, name=f"{nm}sh", tag="stgh", bufs=10)
        sl = tmp.tile([HI, N], F32, name=f"{nm}sl", tag="stgl", bufs=10)
        nc.sync.dma_start(out=sh.rearrange("p (m c) -> p m c", m=2),
                          in_=T[mA:mA + 2, 0:HI, :].transpose([1, 0, 2]))
        nc.sync.dma_start(out=sl, in_=T[mA:mA + 2, HI:N, :])
        stg[nm] = (sh, sl)

    s0h = sp.tile([HI, 2 * N], BF16, tag="s0h")
    s0l = sp.tile([HI, N], BF16, tag="s0l")
    nc.vector.tensor_sub(s0h, stg["d"][0], stg["p"][0])
    nc.gpsimd.tensor_sub(s0l, stg["d"][1], stg["p"][1])
    rh = sp.tile([HI, 2 * N], BF16, tag="rh")
    rl = sp.tile([HI, N], BF16, tag="rl")
    nc.scalar.copy(rh, stg["r"][0])
    nc.gpsimd.tensor_copy(rl, stg["r"][1])
    rbd = sp.tile([HI, HI], BF16, tag="rbd")
    nc.gpsimd.memset(rbd, 0.0)
    nc.gpsimd.tensor_copy(rbd[0:LO, 0:LO], rl[0:LO, HI:N])
    nc.gpsimd.tensor_copy(rbd[LO:HI, LO:HI], rl[LO:HI, HI:N])
    partials = sp.tile([HI, 27], F32, tag="part")
    return {"sh": [s0h], "sl": [s0l], "rh": rh, "rl": rl, "rbd": rbd,
            "part": partials}


def _mm_pair(nc, ps_big, ps_lo, lhs_hi, lhs_lo, bd, rhs_hi, rhs_lo, tagp):
    """One 192x192 @ 192x192 product for both pair mols -> PSUM pair tiles.

    out[p,f] = sum_c lhs[c,p] rhs[c,f] per molecule; lhs must be symmetric
    (we pass S or R directly as lhsT).  bd is the block-diagonal packing of
    the two mols' (c-lo, p-lo) corner chunks so that corner runs as ONE
    matmul over the packed-lo partitions.
    """
    ph = ps_big.tile([HI, 2 * N], F32, name=f"ph_{tagp}", tag="pbig")
    pl = ps_lo.tile([HI, N], F32, name=f"pl_{tagp}", tag="plo")
    for m, c0, p0 in ((0, 0, 0), (1, N, LO)):  # mol A, mol B
        hi_c = lhs_hi[:, c0:c0 + HI]          # lhs cols 0:128 (out rows hi)
        hi_cl = lhs_hi[:, c0 + HI:c0 + N]     # lhs cols 128:192 (out rows lo)
        lo_c = lhs_lo[p0:p0 + LO, 0:HI]
        rhi = rhs_hi[:, c0:c0 + N]
        rlo = rhs_lo[p0:p0 + LO, :]
        nc.tensor.matmul(ph[:, c0:c0 + N], lhsT=hi_c, rhs=rhi,
                         start=True, stop=False)
        nc.tensor.matmul(ph[:, c0:c0 + N], lhsT=lo_c, rhs=rlo,
                         start=False, stop=True)
        nc.tensor.matmul(pl[p0:p0 + LO, :], lhsT=hi_cl, rhs=rhi,
                         start=True, stop=False)
    nc.tensor.matmul(pl[:, :], lhsT=bd, rhs=rhs_lo,
                     start=False, stop=True, skip_group_check=True)
    return ph, pl


def _drain(nc, eng, out, in_):
    if eng == "dve":
        nc.vector.tensor_copy(out, in_)
    else:
        nc.scalar.copy(out, in_)


# engine schedule for the per-level PSUM drains (dve/act only: gpsimd
# cannot touch PSUM)
T_BIG_ENG = ["act", "act", "act", "act"]
S_BIG_ENG = ["act", "act", "act", "act"]
T_SM_ENG = ["dve", "dve", "dve", "dve"]
S_SM_ENG = ["dve", "dve", "dve", "dve"]


def _level(nc, st, pairs, k, sp, tmp, ps_big, ps_lo):
    """Level k: T = S_{k-1} R then S_k = R T for every pair."""
    bds = {}
    for q in pairs:
        s = st[q]
        bd = tmp.tile([HI, HI], BF16, name=f"bdt{k}_{q}", tag="bdt",
                      bufs=BLKP + 2)
        sl = s["sl"][k - 1]
        nc.gpsimd.memset(bd, 0.0)
        nc.gpsimd.tensor_copy(bd[0:LO, 0:LO], sl[0:LO, HI:N])
        nc.gpsimd.tensor_copy(bd[LO:HI, LO:HI], sl[LO:HI, HI:N])
        bds[q] = bd
    tps = {}
    for q in pairs:
        s = st[q]
        tps[q] = _mm_pair(nc, ps_big, ps_lo, s["sh"][k - 1], s["sl"][k - 1],
                          bds[q], s["rh"], s["rl"], f"t{k}_{q}")
    tts = {}
    for q in pairs:
        th = tmp.tile([HI, 2 * N], BF16, name=f"th{k}_{q}", tag="th",
                      bufs=BLKP + 2)
        tl = tmp.tile([HI, N], BF16, name=f"tl{k}_{q}", tag="tl",
                      bufs=BLKP + 2)
        _drain(nc, T_BIG_ENG[k - 1], th, tps[q][0])
        _drain(nc, T_SM_ENG[k - 1], tl, tps[q][1])
        tts[q] = (th, tl)
    sps = {}
    for q in pairs:
        s = st[q]
        sps[q] = _mm_pair(nc, ps_big, ps_lo, s["rh"], s["rl"], s["rbd"],
                          tts[q][0], tts[q][1], f"s{k}_{q}")
    for q in pairs:
        s = st[q]
        skh = sp.tile([HI, 2 * N], BF16, name=f"s{k}h", tag=f"s{k}h",
                      bufs=BLKP + 2)
        skl = sp.tile([HI, N], BF16, name=f"s{k}l", tag=f"s{k}l",
                      bufs=BLKP + 2)
        _drain(nc, S_BIG_ENG[k - 1], skh, sps[q][0])
        _drain(nc, S_SM_ENG[k - 1], skl, sps[q][1])
        s["sh"].append(skh)
        s["sl"].append(skl)


# evens (squares) hi tiles go to ACT; everything else is DVE TTR
def _gram_level(nc, st, pairs, svals, tmp):
    """g[s] = <S_a, S_b> partial sums into partials cols (hi-A: s,
    hi-B: 9+s, lo-pair: 18+s) via fused TTR on DVE / Square-accum on ACT."""
    for s in svals:
        a, bb = G_PAIRS[s]
        for q in pairs:
            stq = st[q]
            part = stq["part"]
            ah, bh = stq["sh"][a], stq["sh"][bb]
            al, bl = stq["sl"][a], stq["sl"][bb]
            if a == bb:
                for m, col in ((0, s), (1, 9 + s)):
                    junk = tmp.tile([HI, N], BF16, name="ja", tag="ja",
                                    bufs=3)
                    nc.scalar.activation(out=junk,
                                         in_=ah[:, m * N:(m + 1) * N],
                                         func=ACTF.Square,
                                         accum_out=part[:, col:col + 1])
            else:
                for m, col in ((0, s), (1, 9 + s)):
                    junk = tmp.tile([HI, N], BF16, name="jd", tag="jd",
                                    bufs=3)
                    nc.vector.scalar_tensor_tensor(
                        out=junk, in0=ah[:, m * N:(m + 1) * N], scalar=1.0,
                        in1=bh[:, m * N:(m + 1) * N],
                        op0=ALU.mult, op1=ALU.mult,
                        accum_out=part[:, col:col + 1])
            junk = tmp.tile([HI, N], BF16, name="jl", tag="jd", bufs=3)
            nc.vector.scalar_tensor_tensor(
                out=junk, in0=al, scalar=1.0, in1=bl,
                op0=ALU.mult, op1=ALU.mult, accum_out=part[:, 18 + s:19 + s])


def _gather(nc, st, pairs, C, tail, ps_tl):
    """Cross-partition reduce all pairs' partials into g_sb [32, 9]."""
    cb1, cb2 = C["cb1"], C["cb2"]
    gath = ps_tl.tile([2 * BLKP, 12], F32, tag="gath")
    nmm = 3 * len(pairs)
    i = 0
    for j, q in enumerate(pairs):
        part = st[q]["part"]
        rA = 2 * j
        W = 2 * BLKP
        for lhsT, rhs in (
            (cb1[:, W - 1 - rA:2 * W - 1 - rA], part[:, 0:9]),
            (cb1[:, W - 2 - rA:2 * W - 2 - rA], part[:, 9:18]),
            (cb2[:, W - 1 - rA:2 * W - 1 - rA], part[:, 18:27]),
        ):
            nc.tensor.matmul(gath[:, 0:9], lhsT=lhsT, rhs=rhs,
                             start=(i == 0), stop=(i == nmm - 1))
            i += 1
    g_sb = tail.tile([2 * BLKP, 9], F32, tag="g_sb")
    nc.vector.tensor_copy(g_sb, gath[:, 0:9])
    return g_sb


def _tail(nc, pairs, C, tail, ps_tl, g_sb):
    """Batched 4x4 solve from g, then broadcast -y to [128, *] columns."""
    nm = 2 * BLKP  # 32 molecules
    g = g_sb
    # Hankel assembly: h[s] = g[s] - 2 g[s+1] + g[s+2]; rhs c = diff(g)
    hs = tail.tile([nm, 7], F32, tag="hs")
    hm = tail.tile([nm, 7], F32, tag="hm")
    h = tail.tile([nm, 7], F32, tag="h")
    nc.vector.tensor_add(hs, g[:, 0:7], g[:, 2:9])
    nc.vector.tensor_scalar(out=hm, in0=g[:, 1:8], scalar1=-2.0, scalar2=None,
                            op0=ALU.mult)
    nc.vector.tensor_add(h, hs, hm)
    sv = tail.tile([nm, 14], F32, tag="sv")
    nc.vector.tensor_copy(sv[:, 0:4], h[:, 0:4])
    nc.vector.tensor_copy(sv[:, 4:7], h[:, 2:5])
    nc.vector.tensor_copy(sv[:, 7:9], h[:, 4:6])
    nc.vector.tensor_copy(sv[:, 9:10], h[:, 6:7])
    nc.vector.tensor_sub(sv[:, 10:14], g[:, 1:5], g[:, 0:4])

    ysb = _solve(nc, sv, tail, nm)
    ysn = tail.tile([nm, 4], F32, tag="ysn")
    nc.vector.tensor_scalar(out=ysn, in0=ysb, scalar1=-1.0, scalar2=None,
                            op0=ALU.mult)

    # broadcast -y to all partitions: cols 4m:(4m+4) per mol; cols
    # 128+4q:(128+4q+4) carry the packed-lo per-partition-half values.
    ones, selp = C["ones"], C["selp"]
    ysn_b = ysn.unsqueeze(1)
    yp = tail.tile([nm, 4 * nm], F32, tag="yp")
    nc.vector.tensor_mul(
        yp.rearrange("p (m i) -> p m i", i=4),
        C["mask32"].rearrange("p (m i) -> p m i", i=4),
        ysn_b.broadcast_to([nm, nm, 4]))
    yq = tail.tile([nm, 4 * BLKP], F32, tag="yq")
    nc.vector.tensor_mul(
        yq.rearrange("p (m i) -> p m i", i=4),
        C["mask2"].rearrange("p (m i) -> p m i", i=4),
        ysn_b.broadcast_to([nm, BLKP, 4]))
    ybp = ps_tl.tile([HI, N], F32, tag="ybp")
    nc.tensor.matmul(ybp[:, 0:4 * nm], lhsT=ones[0:nm, 0:HI], rhs=yp,
                     start=True, stop=True)
    nc.tensor.matmul(ybp[:, 4 * nm:4 * nm + 4 * BLKP],
                     lhsT=selp[0:nm, 0:HI],
                     rhs=yq, start=True, stop=True)
    ybc = tail.tile([HI, N], F32, tag="ybc")
    nc.scalar.copy(ybc, ybp)
    return ybc


def _solve(nc, sv, tail, nm):
    """Batched symmetric 4x4 Gauss elimination on [nm,1] column APs.

    sv cols: 0:a 1:b 2:c 3:d | 4:e 5:f 6:g | 7:h 8:i | 9:j | 10..13 r0..r3.
    Mirrors solve_batched_np (validated offline).
    """
    pp = tail.tile([nm, 4], F32, tag="pp")
    l3 = tail.tile([nm, 3], F32, tag="l3")
    tt = tail.tile([nm, 3], F32, tag="tt")
    ysb = tail.tile([nm, 4], F32, tag="ysb")

    ts = nc.vector.tensor_scalar
    sub = nc.vector.tensor_sub
    rec = nc.vector.reciprocal

    def upd(dst, src, scal, w=1):
        ts(out=tt[:, 0:w], in0=src, scalar1=scal, scalar2=None, op0=ALU.mult)
        sub(dst, dst, tt[:, 0:w])

    rec(pp[:, 0:1], sv[:, 0:1])
    ts(out=l3, in0=sv[:, 1:4], scalar1=pp[:, 0:1], scalar2=None, op0=ALU.mult)
    upd(sv[:, 4:7], l3, sv[:, 1:2], 3)          # (e,f,g) -= l*b
    upd(sv[:, 7:9], l3[:, 1:3], sv[:, 2:3], 2)  # (h,i) -= (l2,l3)*c
    upd(sv[:, 9:10], l3[:, 2:3], sv[:, 3:4])    # j -= l3*d
    upd(sv[:, 11:14], l3, sv[:, 10:11], 3)      # (r1,r2,r3) -= l*r0
    rec(pp[:, 1:2], sv[:, 4:5])
    ts(out=l3[:, 1:3], in0=sv[:, 5:7], scalar1=pp[:, 1:2], scalar2=None,
       op0=ALU.mult)
    upd(sv[:, 7:9], l3[:, 1:3], sv[:, 5:6], 2)
    upd(sv[:, 9:10], l3[:, 2:3], sv[:, 6:7])
    upd(sv[:, 12:14], l3[:, 1:3], sv[:, 11:12], 2)
    rec(pp[:, 2:3], sv[:, 7:8])
    ts(out=l3[:, 2:3], in0=sv[:, 8:9], scalar1=pp[:, 2:3], scalar2=None,
       op0=ALU.mult)
    upd(sv[:, 9:10], l3[:, 2:3], sv[:, 8:9])
    upd(sv[:, 13:14], l3[:, 2:3], sv[:, 12:13])
    rec(pp[:, 3:4], sv[:, 9:10])
    ts(out=ysb[:, 3:4], in0=sv[:, 13:14], scalar1=pp[:, 3:4], scalar2=None,
       op0=ALU.mult)
    upd(sv[:, 12:13], sv[:, 8:9], ysb[:, 3:4])
    ts(out=ysb[:, 2:3], in0=sv[:, 12:13], scalar1=pp[:, 2:3], scalar2=None,
       op0=ALU.mult)
    upd(sv[:, 11:12], sv[:, 5:6], ysb[:, 2:3])
    upd(sv[:, 11:12], sv[:, 6:7], ysb[:, 3:4])
    ts(out=ysb[:, 1:2], in0=sv[:, 11:12], scalar1=pp[:, 1:2], scalar2=None,
       op0=ALU.mult)
    upd(sv[:, 10:11], sv[:, 1:2], ysb[:, 1:2])
    upd(sv[:, 10:11], sv[:, 2:3], ysb[:, 2:3])
    upd(sv[:, 10:11], sv[:, 3:4], ysb[:, 3:4])
    ts(out=ysb[:, 0:1], in0=sv[:, 10:11], scalar1=pp[:, 0:1], scalar2=None,
       op0=ALU.mult)
    return ysb


def _combo(nc, stq, j, ybc, tmp):
    """acc = sum_I (-y_I) S_I via a fused scale-accumulate (STT) chain."""
    mA, mB = 2 * j, 2 * j + 1
    ah = tmp.tile([HI, 2 * N], F32, name="acch", tag="acch", bufs=4)
    al = tmp.tile([HI, N], F32, name="accl", tag="accl", bufs=4)
    for m, c0 in ((mA, 0), (mB, N)):
        u0 = tmp.tile([HI, N], BF16, name="cu0", tag="cu", bufs=6)
        nc.vector.tensor_scalar(out=u0, in0=stq["sh"][0][:, c0:c0 + N],
                                scalar1=ybc[:, 4 * m:4 * m + 1],
                                scalar2=None, op0=ALU.mult)
        for I in (1, 2):
            u1 = tmp.tile([HI, N], BF16, name="cu1", tag="cu", bufs=6)
            nc.vector.scalar_tensor_tensor(
                out=u1, in0=stq["sh"][I][:, c0:c0 + N],
                scalar=ybc[:, 4 * m + I:4 * m + I + 1], in1=u0,
                op0=ALU.mult, op1=ALU.add)
            u0 = u1
        nc.vector.scalar_tensor_tensor(
            out=ah[:, c0:c0 + N], in0=stq["sh"][3][:, c0:c0 + N],
            scalar=ybc[:, 4 * m + 3:4 * m + 4], in1=u0,
            op0=ALU.mult, op1=ALU.add)
    u0 = tmp.tile([HI, N], BF16, name="cul0", tag="cu", bufs=6)
    nc.vector.tensor_scalar(out=u0, in0=stq["sl"][0],
                            scalar1=ybc[:, PAIR_OFF + 4 * j:PAIR_OFF + 4 * j + 1],
                            scalar2=None, op0=ALU.mult)
    for I in (1, 2):
        u1 = tmp.tile([HI, N], BF16, name="cul1", tag="cu", bufs=6)
        nc.vector.scalar_tensor_tensor(
            out=u1, in0=stq["sl"][I],
            scalar=ybc[:, PAIR_OFF + 4 * j + I:PAIR_OFF + 4 * j + I + 1], in1=u0,
            op0=ALU.mult, op1=ALU.add)
        u0 = u1
    nc.vector.scalar_tensor_tensor(
        out=al, in0=stq["sl"][3],
        scalar=ybc[:, PAIR_OFF + 4 * j + 3:PAIR_OFF + 4 * j + 4], in1=u0,
        op0=ALU.mult, op1=ALU.add)
    stq["acc"] = (ah, al)


def _store(nc, OUT, q, stq):
    mA = 2 * q
    ah, al = stq["acc"]
    nc.sync.dma_start(out=OUT[mA:mA + 2, 0:HI, :].transpose([1, 0, 2]),
                      in_=ah.rearrange("p (m c) -> p m c", m=2))
    nc.sync.dma_start(out=OUT[mA:mA + 2, HI:N, :], in_=al)


_NC_CACHE = None


def _get_nc():
    global _NC_CACHE
    if _NC_CACHE is None:
        _NC_CACHE = build_core_kernel()
    return _NC_CACHE


def kernel(D, P, R, max_rank=4, _trace=False):
    D = np.ascontiguousarray(D, dtype=np.float32)
    P = np.ascontiguousarray(P, dtype=np.float32)
    R = np.ascontiguousarray(R, dtype=np.float32)
    nc = _get_nc()
    in_maps = []
    for i in range(NCORES):
        sl = slice(i * MPC, (i + 1) * MPC)
        in_maps.append({"D": D[sl], "P": P[sl], "Rm": R[sl]})
    res = run_bass_kernel_spmd(nc, in_maps, core_ids=list(range(NCORES)),
                               trace=_trace)
    out = np.concatenate([r["OUT"] for r in res.results], axis=0)
    if _trace:
        kernel.last_exec_time_ns = res.exec_time_ns
        kernel.last_trace = res.instructions_and_trace
    return out


if __name__ == "__main__":
    import tempfile
    from concourse.bass_utils import compile_bass_kernel
    nc = build_core_kernel()
    print("build OK")
    if "--compile" in sys.argv:
        td = tempfile.mkdtemp()
        print("NEFF:", compile_bass_kernel(nc, td))
